# revision 2
# baseline (speedup 1.0000x reference)
"""Neural CDE encoder kernel for 8 Trainium2 NeuronCores — collective-free.

Math (validated by the previous kernel): all 20 Euler times t_k = 0.05*k lie
in spline interval 0, so dX_k[b] = base_b + mcoef_k * m_b with
base_b = y1 - y0 and m_b = M1 (second-derivative row), mcoef_k = t_k^2/2 - 1/6.
Therefore each Euler step is linear in z with a per-batch operator:

    z_{k+1}[b] = z_k[b] + (P_b + mcoef_k * Q_b) z_k[b]
    P_b = dt * sum_d base[b,d] W_d,   Q_b = dt * sum_d m[b,d] W_d
    (W_d[h',h] = W_lin[h'*D+d, h], dt = 0.05)

Sharding: pure data-parallel over batch (8 of 64 trajectories per core) — no
collectives at all.  Each core builds P_b^T, Q_b^T for its 8 b's (bf16,
18.9 MB resident in SBUF) by streaming the host-transposed weight tensor
wm[t,d,h',hi] = W_lin[h'*D+d, t*128+hi] once through the PE (stationary =
(64,128) W slices — FWL streams them at 4 cols/cycle), then runs the 20 Euler
steps as batched 768x768 mat-vecs: stationary = 128x128 tiles of P^T/Q^T
(LDWEIGHTS+FWL, ~29ns/tile), moving = z column, accumulating u_P, u_Q in PSUM
(128, 8b).  DVE updates a float32 master z and casts a bf16 copy for the next
step's moving operand.  z is projected through W_out^T each step; a final
(21->128) interpolation matmul produces each core's (8, 128, 256) output
slice; the host concatenates over batch.
"""

import numpy as np

B, L, D, H, O = 64, 128, 64, 768, 256
NS = 20              # Euler steps
NC = 8               # cores
BL = B // NC         # 8 trajectories per core
KT = H // 128        # 6 h-tiles
DT = 0.05

_prog_cache = {}


def _host_constants():
    grid = (np.arange(NS + 1, dtype=np.float32) * np.float32(DT)).astype(np.float32)
    grid[-1] = np.float32(1.0)
    tk = (grid[:-1]).astype(np.float64)
    mcoef = (tk * tk / 2.0 - 1.0 / 6.0).astype(np.float32)

    # w over L such that M1 = <w, y> (row 0 of tridiagonal inverse, 2nd-diff'd)
    n = L - 2
    A = 4.0 * np.eye(n) + np.eye(n, k=1) + np.eye(n, k=-1)
    r0 = np.linalg.solve(A, np.eye(n)[:, 0])
    w = np.zeros(L, dtype=np.float64)
    w[0:n] += 6.0 * r0
    w[1:n + 1] += -12.0 * r0
    w[2:n + 2] += 6.0 * r0

    # Interp matrix J (L, NS+1): out_z[l] = sum_k J[l,k] z_grid[k]
    ts = np.linspace(0.0, 1.0, L, dtype=np.float32)
    j = np.clip(np.searchsorted(grid, ts, side="right") - 1, 0, NS - 1)
    wl = ((ts - grid[j]) / (grid[j + 1] - grid[j])).astype(np.float32)
    J = np.zeros((L, NS + 1), dtype=np.float32)
    J[np.arange(L), j] += 1.0 - wl
    J[np.arange(L), j + 1] += wl
    return mcoef, w.astype(np.float64), J.T.copy()  # JT (21, 128)


def _build_program(mcoef, has_blin):
    import concourse.bacc as bacc
    import concourse.mybir as mybir
    import concourse.tile as tile

    f32 = mybir.dt.float32
    bf16 = mybir.dt.bfloat16
    ADD = mybir.AluOpType.add
    MUL = mybir.AluOpType.mult

    nc = bacc.Bacc("TRN2", target_bir_lowering=False, debug=False,
                   num_devices=NC)

    wm_d = nc.dram_tensor("wm", [KT, H // 128, 128, 64 * 128], bf16,
                          kind="ExternalInput")
    bs_d = nc.dram_tensor("bs", [D, 2 * BL], bf16, kind="ExternalInput")
    z0_d = nc.dram_tensor("z0t", [H, BL], f32, kind="ExternalInput")
    wot_d = nc.dram_tensor("wot", [H, O], bf16, kind="ExternalInput")
    jt_d = nc.dram_tensor("jt", [NS + 1, L], bf16, kind="ExternalInput")
    if has_blin:
        c12_d = nc.dram_tensor("c12t", [H, 2 * BL], f32, kind="ExternalInput")
    out_d = nc.dram_tensor("out", [BL, L, O], f32, kind="ExternalOutput")

    GRP = 128           # h' per wm DMA chunk (two 64-h' halves on d-blocks)
    NG = H // GRP       # 6 chunks per h-tile

    with tile.TileContext(nc) as tc:
        with (
            tc.tile_pool(name="pers", bufs=1) as pers,
            tc.tile_pool(name="dram", bufs=1, space="DRAM") as dram,
        ):
            # ---- persistent tiles ------------------------------------------
            pt = [pers.tile([128, 2 * BL * H], bf16, tag=f"pt{t}",
                            name=f"pt{t}") for t in range(KT)]
            sb_bs = pers.tile([128, 2 * BL], bf16, tag="bs")
            nc.sync.dma_start(sb_bs[:D, :], bs_d[:])
            nc.sync.dma_start(sb_bs[D:, :], bs_d[:])
            sb_wot = pers.tile([128, KT * O], bf16, tag="wot")
            nc.sync.dma_start(
                sb_wot[:].rearrange("p (t o) -> p t o", t=KT),
                wot_d[:].rearrange("(t p) o -> p t o", t=KT))
            sb_jt = pers.tile([NS + 1, L], bf16, tag="jt")
            nc.sync.dma_start(sb_jt[:], jt_d[:])
            y_store = pers.tile([BL, (NS + 1) * O], bf16, tag="ystore")
            zf = [pers.tile([128, BL], f32, tag=f"zf{i}", name=f"zf{i}")
                  for i in range(2 * KT)]
            zb = [pers.tile([128, BL], bf16, tag=f"zb{i}", name=f"zb{i}")
                  for i in range(2 * KT)]
            if has_blin:
                sb_c12 = pers.tile([128, KT * 2 * BL], f32, tag="c12")
                nc.sync.dma_start(
                    sb_c12[:].rearrange("p (t c) -> p t c", t=KT),
                    c12_d[:].rearrange("(t p) c -> p t c", t=KT))

            for t in range(KT):
                nc.sync.dma_start(zf[t][:], z0_d[128 * t:128 * (t + 1), :])
                nc.scalar.copy(zb[t][:], zf[t][:])

            # ---- build P^T/Q^T, with Euler step 0 interleaved ----------
            # g-outer: chunk (t, g) fills pt[t] h'-range [128g, 128(g+1)) =
            # h'-tile m=g, so after each g all stationaries for step-0's m=g
            # mat-vecs exist and run during the next group's DMA wait.
            y_d = dram.tile([NS + 1, BL * O], bf16)
            HM = KT // 2
            with (
                tc.tile_pool(name="wmp", bufs=3) as wmp,
                tc.tile_pool(name="psb", bufs=2, space="PSUM") as psb,
                tc.tile_pool(name="psu", bufs=2, space="PSUM") as psu,
                tc.tile_pool(name="psj", bufs=2, space="PSUM") as psj,
                tc.tile_pool(name="work", bufs=2) as work,
            ):
                def project(k, zbk):
                    psp = psj.tile([BL, O], f32, tag="psj", name="psp")
                    for t in range(KT):
                        nc.tensor.matmul(psp[:], zbk[t][:],
                                         sb_wot[:, O * t:O * (t + 1)],
                                         start=(t == 0), stop=(t == KT - 1))
                    ys = y_store[:, O * k:O * (k + 1)]
                    nc.vector.tensor_copy(ys, psp[:])
                    nc.sync.dma_start(
                        y_d[k].rearrange("(b o) -> b o", b=BL), ys)

                def step_mms(u_h, k, mh, mi):
                    cur = (k % 2) * KT
                    m = mh * HM + mi
                    for mat in range(2):
                        for b in range(BL):
                            col = (mat * BL + b) * H + 128 * m
                            uc = (mat * HM + mi) * BL + b
                            for t in range(KT):
                                nc.tensor.matmul(
                                    u_h[:, uc:uc + 1],
                                    pt[t][:, col:col + 128],
                                    zb[cur + t][:, b:b + 1],
                                    start=(t == 0), stop=(t == KT - 1))

                def step_dve(u_h, k, mh, mi):
                    cur, nxt = (k % 2) * KT, ((k + 1) % 2) * KT
                    mck = float(mcoef[k])
                    m = mh * HM + mi
                    up_s = u_h[:, mi * BL:(mi + 1) * BL]
                    uq_s = u_h[:, (HM + mi) * BL:(HM + mi + 1) * BL]
                    tmp = work.tile([128, BL], f32, tag="tmp", name="tmp")
                    nc.vector.scalar_tensor_tensor(
                        tmp[:], uq_s, mck, zf[cur + m][:], op0=MUL, op1=ADD)
                    if has_blin:
                        c1s = sb_c12[:, 2 * BL * m:2 * BL * m + BL]
                        c2s = sb_c12[:, 2 * BL * m + BL:2 * BL * (m + 1)]
                        tmp2 = work.tile([128, BL], f32, tag="tmp2",
                                         name="tmp2")
                        nc.vector.scalar_tensor_tensor(
                            tmp2[:], c2s, mck, c1s, op0=MUL, op1=ADD)
                        nc.vector.tensor_tensor(tmp[:], tmp[:], tmp2[:], ADD)
                    nc.vector.tensor_tensor(zf[nxt + m][:], tmp[:], up_s, ADD)
                    nc.scalar.copy(zb[nxt + m][:], zf[nxt + m][:])

                project(0, zb[:KT])

                u0 = [None, None]
                for g in range(NG):
                    if g % HM == 0:
                        u0[g // HM] = psu.tile([128, 2 * HM * BL], f32,
                                               tag=f"u{g // HM}", name="u0h")
                    for t in range(KT):
                        pt3 = pt[t][:].rearrange("p (c w) -> p c w", c=2 * BL)
                        ch = wmp.tile([128, GRP // 2 * 128], bf16, tag="wm",
                                      name="ch")
                        half = D * 128 // 2
                        nc.sync.dma_start(ch[:, :half], wm_d[t, g][:, :half])
                        nc.scalar.dma_start(ch[:, half:], wm_d[t, g][:, half:])
                        for jb in range(GRP // 8):
                            ps = psb.tile([128, 128], f32, tag="psb",
                                          name="ps")
                            for j in range(8):
                                hp = jb * 8 + j
                                hf, hpl = hp // 64, hp % 64
                                nc.tensor.matmul(
                                    ps[:, 16 * j:16 * (j + 1)],
                                    ch[64 * hf:64 * (hf + 1),
                                       128 * hpl:128 * (hpl + 1)],
                                    sb_bs[64 * hf:64 * (hf + 1), :],
                                    start=True, stop=True)
                            src = ps[:].rearrange("p (j c) -> p c j", j=8)
                            off = GRP * g + 8 * jb
                            nc.vector.tensor_copy(pt3[:, :, off:off + 8], src)
                    # step-0 mat-vecs for h'-tile m=g ride the DMA shadow
                    step_mms(u0[g // HM], 0, g // HM, g % HM)
                    if g % HM == HM - 1:
                        for mi in range(HM):
                            step_dve(u0[g // HM], 0, g // HM, mi)

                project(1, zb[KT:])

                # ---- Euler steps 1..NS-1 -------------------------------
                for k in range(1, NS):
                    for mh in range(2):
                        u_h = psu.tile([128, 2 * HM * BL], f32, tag=f"u{mh}",
                                       name=f"u{mh}")
                        for mi in range(HM):
                            step_mms(u_h, k, mh, mi)
                        for mi in range(HM):
                            step_dve(u_h, k, mh, mi)
                    nxt = ((k + 1) % 2) * KT
                    project(k + 1, zb[nxt:nxt + KT])

            # ---- interpolation + output ------------------------------------
            with (
                tc.tile_pool(name="tail", bufs=2) as tail,
                tc.tile_pool(name="psi", bufs=2, space="PSUM") as psi,
            ):
                yk = tail.tile([NS + 1, BL * O], bf16, tag="yk")
                nc.sync.dma_start(yk[:], y_d[:])
                out3 = out_d.ap().rearrange("b l o -> l b o")
                CB = 512 // O  # batch elems per 512-col chunk
                for c in range(BL * O // 512):
                    pso = psi.tile([L, 512], f32, tag="psi", name="pso")
                    nc.tensor.matmul(pso[:], sb_jt[:],
                                     yk[:, 512 * c:512 * (c + 1)])
                    sbo = tail.tile([L, 512], f32, tag="sbo", name="sbo")
                    nc.vector.tensor_copy(sbo[:], pso[:])
                    nc.sync.dma_start(
                        out3[:, CB * c:CB * (c + 1), :],
                        sbo[:].rearrange("l (b o) -> l b o", o=O))

    nc.compile()
    return nc


def _make_in_maps(inputs):
    import ml_dtypes

    bf16 = ml_dtypes.bfloat16
    traj = np.asarray(inputs["traj"], dtype=np.float32)
    W_lin = np.asarray(inputs["W_lin"], dtype=np.float32)
    b_lin = np.asarray(inputs["b_lin"], dtype=np.float32)
    W_out = np.asarray(inputs["W_out"], dtype=np.float32)
    b_out = np.asarray(inputs["b_out"], dtype=np.float32)
    W_z0 = np.asarray(inputs["W_z0"], dtype=np.float32)
    b_z0 = np.asarray(inputs["b_z0"], dtype=np.float32)

    mcoef, w, JT = _host_constants()
    has_blin = bool(np.any(b_lin))

    base = traj[:, 1, :] - traj[:, 0, :]                      # (B, D)
    m = np.einsum("l,bld->bd", w, traj.astype(np.float64)).astype(np.float32)
    z0 = traj[:, 0, :] @ W_z0.T + b_z0                        # (B, H) f32

    # wm[t, g, 64*j+d, hp*128+hi] = W_lin[(128g+64j+hp)*D + d, 128t + hi]
    # i.e. each (t, g) block is exactly the SBUF chunk image for the build.
    wm = np.ascontiguousarray(
        W_lin.reshape(KT, 2, D, D, KT, 128).transpose(4, 0, 1, 3, 2, 5)
        .reshape(KT, KT, 128, D * 128)).astype(bf16)
    wot = np.ascontiguousarray(W_out.T).astype(bf16)
    jt = JT.astype(bf16)

    if has_blin:
        Blin = b_lin.reshape(H, D)
        c1 = DT * base @ Blin.T                               # (B, H)
        c2 = DT * m @ Blin.T

    in_maps = []
    for i in range(NC):
        sl = slice(BL * i, BL * (i + 1))
        bs = np.concatenate([DT * base[sl].T, DT * m[sl].T], axis=1)  # (D, 16)
        mday = dict(
            wm=wm,
            bs=np.ascontiguousarray(bs).astype(bf16),
            z0t=np.ascontiguousarray(z0[sl].T),
            wot=wot,
            jt=jt,
        )
        if has_blin:
            mday["c12t"] = np.ascontiguousarray(
                np.concatenate([c1[sl].T, c2[sl].T], axis=1))
        in_maps.append(mday)
    return in_maps, has_blin, b_out


def kernel(**inputs):
    from concourse.bass_utils import run_bass_kernel_spmd

    in_maps, has_blin, b_out = _make_in_maps(inputs)
    if has_blin not in _prog_cache:
        mcoef, _, _ = _host_constants()
        _prog_cache[has_blin] = _build_program(mcoef, has_blin)
    nc = _prog_cache[has_blin]

    res = run_bass_kernel_spmd(nc, in_maps, core_ids=list(range(NC)))
    out = np.concatenate([r["out"] for r in res.results], axis=0)
    if np.any(b_out):
        out = out + b_out[None, None, :]
    return out
